# revision 2
# baseline (speedup 1.0000x reference)
"""Trainium2 Bass kernel for nn_MixedOp_35562329211102.

Computes FM[b,c] = expm( sum_o weights[o] * logm( W[o,c]^T x[b,c] W[o,c] ) )
for x: [256,16,64,64] SPD, W: [6,16,64,32], weights: [6] (simplex).

logm via a dyadic squaring chain: H_0 = I - Y/theta, H_{j+1} = H_j^2.
log(Y) = log(theta) + log(I - H_0) ~ sum_j c_j H_j, with coefficients from a
Lawson minimax fit weighted by (1-h) -- errors at tiny eigenvalues of Y are
crushed by the final expm, so the fit spends its budget where it matters.
That lets K_SQ drop from 13 to 8 at equal end-to-end error (fp16-sim
rel_l2 ~3.8e-3 vs gate 2e-2).

Squarings run as block-diagonal quad matmuls: the stationary operand is a
128x128 block-diagonal matrix holding 4 independent 32x32 H's (one per
partition group = channel), the moving operand is the natural stacked
[128, 32] tile.  out = B^T @ D yields all 4 squares in stacked layout with
one LDWEIGHTS (128 cols -> fast-weight-load eligible) + one matmul (N=32)
instead of 4 LDW + 4 MM.  Block-diagonal operands are maintained by 3-engine
scatter copies (gpsimd/vector/scalar) into pre-zeroed persistent tiles.

expm via scaling-squaring: X = M/8, degree-6 Taylor (Paterson-Stockmeyer),
then 3 squarings, same quad-matmul scheme.

Sharding: data-parallel over batch B across 8 cores (32 batches/core).
"""

import numpy as np

import concourse.bass as bass
from concourse import bacc
import concourse.mybir as mybir
from concourse.bass import AP
from concourse.tile import TileContext

FP = mybir.dt.float32
HP = mybir.dt.float16
AOP = mybir.AluOpType

THETA = 9.0
LOGTHETA = 2.1972245773
K_SQ = 8
# (1-h)-weighted Lawson fit of log(1-h) on h in [0.0626, 0.999817]:
# log(1-h) ~ HCOEF[0] + sum_{j=0..K_SQ} HCOEF[1+j] * h^(2^j)
HCOEF = [-0.00033517, -0.99035730, -0.58854727, -0.77016997, -0.59288672,
         -0.87173622, -0.32950715, -1.43648230, 0.64457418, -2.38238148]
EXPC = [1.0, 1.0, 0.5, 1.0 / 6, 1.0 / 24, 1.0 / 120, 1.0 / 720]

C, O, D, DIN = 16, 6, 32, 64
NCORES = 8

WT_KINDS = [f'H{j}' for j in range(K_SQ + 1)]
WT_NCOL = len(WT_KINDS) * O


def host_wtab(weights: np.ndarray) -> np.ndarray:
    """[128, WT_NCOL] per-partition scalar table: w[o]/8 * c_j."""
    w8 = weights.astype(np.float64) / 8.0
    cols = [w8 * HCOEF[1 + j] for j in range(K_SQ + 1)]
    row = np.concatenate(cols)
    return np.tile(row[None, :], (128, 1)).astype(np.float32)


def host_idt() -> np.ndarray:
    """[128, 32]: 4 stacked 32x32 identities."""
    return np.tile(np.eye(D, dtype=np.float32), (4, 1))


def _bc(t, nblk):
    """broadcast a [128, D] tile AP over nblk column blocks -> [128, nblk, D]."""
    a = t[:, :]
    return AP(a.tensor, a.offset, [list(a.ap[0]), [0, nblk], [1, D]])


def _blk(ap, nblk):
    """view a [128, nblk*D] AP as [128, nblk, D]."""
    return ap.rearrange("p (n j) -> p n j", n=nblk)


def build_nc(b_loc=32, bchunk=8, replicate=1):
    nchunk = b_loc // bchunk
    nb = bchunk * D          # stage2 N per (o,c)
    ncols = 4 * bchunk * D   # stacked tile width (32 col-blocks of 32)
    nblk = 4 * bchunk        # 32x32 col-blocks per stacked tile
    bcols = nblk * 128       # block-diag tile width

    nc = bacc.Bacc("TRN2")
    x = nc.dram_tensor("x", [b_loc, C, DIN, DIN], FP, kind="ExternalInput")
    Wt = nc.dram_tensor("W", [O, C, DIN, D], FP, kind="ExternalInput")
    wtab_d = nc.dram_tensor("wtab", [128, WT_NCOL], FP, kind="ExternalInput")
    idt_d = nc.dram_tensor("idt", [128, D], FP, kind="ExternalInput")
    out = nc.dram_tensor("out", [b_loc, C, D, D], FP, kind="ExternalOutput")

    with TileContext(nc) as tc, (
        tc.tile_pool(name="consts", bufs=1)) as consts, (
        tc.tile_pool(name="xp", bufs=4)) as xp, (
        tc.tile_pool(name="vp", bufs=2)) as vp, (
        tc.tile_pool(name="hp", bufs=10)) as hpp, (
        tc.tile_pool(name="ct", bufs=7)) as ctp, (
        tc.tile_pool(name="outp", bufs=2)) as outp, (
        tc.tile_pool(name="xaccp", bufs=2)) as xaccp, (
        tc.tile_pool(name="s1ps", bufs=1, space="PSUM")) as s1psp, (
        tc.tile_pool(name="s2ps", bufs=1, space="PSUM")) as s2psp, (
        tc.tile_pool(name="wkps", bufs=3, space="PSUM")) as wkps:

        # ---- constants ----
        w1t = []
        for cp in range(C // 2):
            tf = consts.tile([128, O * D], FP, tag=f"w1f_{cp}")
            for e in range(2):
                dst = tf[64 * e:64 * (e + 1), :].rearrange("p (o j) -> p o j", o=O)
                src = Wt[:, 2 * cp + e, :, :].rearrange("o p j -> p o j")
                nc.sync.dma_start(dst, src)
            th = consts.tile([128, O * D], HP, tag=f"w1_{cp}")
            nc.vector.tensor_copy(th[:, :], tf[:, :])
            w1t.append(th)
        wtab = consts.tile([128, WT_NCOL], FP, tag="wtab", name="wtab")
        nc.sync.dma_start(wtab[:, :], wtab_d[:, :])
        idt = consts.tile([128, D], FP, tag="idt", name="idt")
        nc.sync.dma_start(idt[:, :], idt_d[:, :])
        cid = {}
        for k in (0, 3):
            t = consts.tile([128, D], HP, tag=f"cid{k}")
            nc.vector.tensor_scalar_mul(t[:, :], idt[:, :], float(EXPC[k]))
            cid[k] = t

        # persistent block-diagonal stationary tiles (4 slots), pre-zeroed
        # once; scatter copies only ever write the diagonal 32x32 blocks, so
        # the off-diagonal zeros persist across reuse.
        btiles = []
        for s in range(4):
            bt = consts.tile([128, bcols], HP, tag=f"bdiag{s}", name=f"bdiag{s}")
            nc.gpsimd.memset(bt[:, :], 0.0)
            btiles.append(bt)

        def wap(kind, o):
            i = WT_KINDS.index(kind) * O + o
            return wtab[:, i:i + 1]

        def bbuild(bt, src):
            """scatter stacked [128, ncols] -> diag blocks of [128, bcols]."""
            engines = [nc.gpsimd, nc.gpsimd, nc.vector, nc.scalar]
            for g in range(4):
                s = src[32 * g:32 * (g + 1), :].rearrange(
                    "p (n j) -> p n j", n=nblk)
                da = bt[32 * g:32 * (g + 1), :]
                dst = AP(da.tensor, da.offset + 32 * g,
                         [list(da.ap[0]), [128, nblk], [1, 32]])
                eng = engines[g]
                if eng is nc.scalar:
                    eng.copy(dst, s)
                else:
                    eng.tensor_copy(dst, s)

        def bwave(ps, bt, src):
            """ps[:, blk*32:+32] = bt[:, blk*128:+128]^T @ src[:, blk*32:+32]
            for all col-blocks: 4 squares per matmul via block-diag weights."""
            for blk in range(nblk):
                nc.tensor.matmul(ps[:, blk * 32:(blk + 1) * 32],
                                 bt[:, blk * 128:(blk + 1) * 128],
                                 src[:, blk * 32:(blk + 1) * 32])

        for _rep in range(replicate):
          for ch in range(nchunk):
            if True:
                Xps = xaccp.tile([128, ncols], FP, tag="xacc", name="xacc")
                nc.vector.memset(Xps[:, :], 0.0)
                hog = [hpp.tile([128, ncols], HP, tag="hog", name="hog")
                       for _ in range(O)]

                # ===== phase A: BiMap + H0 =====
                if True:
                    for q in range(4):
                        vt = vp.tile([128, 2 * O * nb], HP, tag="v", name="v")
                        for cp in (2 * q, 2 * q + 1):
                            e = cp % 2
                            # one bulk DMA + cast for all bchunk batches
                            xf = xp.tile([128, bchunk * DIN], FP, tag="xf",
                                         name="xf")
                            xa = x[:, :, :, :]
                            xsrc = AP(
                                xa.tensor,
                                (ch * bchunk) * C * DIN * DIN
                                + 2 * cp * DIN * DIN,
                                [[DIN * DIN, 2], [DIN, DIN],
                                 [C * DIN * DIN, bchunk], [1, DIN]])
                            nc.sync.dma_start(
                                xf[:, :].rearrange("p (b j) -> p b j",
                                                   b=bchunk), xsrc)
                            xt = xp.tile([128, bchunk * DIN], HP, tag="xt",
                                         name="xt")
                            nc.scalar.copy(xt[:, :], xf[:, :])
                            for bb in range(bchunk):
                                ps1 = s1psp.tile([128, O * D], FP, tag="s1",
                                                 name="s1")
                                xs_ = xt[:, bb * DIN:(bb + 1) * DIN]
                                nc.tensor.matmul(ps1[0:64, :], xs_[0:64, :],
                                                 w1t[cp][0:64, :],
                                                 tile_position=(0, 0))
                                nc.tensor.matmul(ps1[64:128, :], xs_[64:128, :],
                                                 w1t[cp][64:128, :],
                                                 tile_position=(64, 64))
                                # scatter V into o-major layout
                                src = ps1[:, :].rearrange("p (o j) -> p o j", o=O)
                                va = vt[:, :]
                                dst = AP(va.tensor,
                                         va.offset + e * O * nb + bb * D,
                                         [list(va.ap[0]), [nb, O], [1, D]])
                                if bb % 2 == 0:
                                    nc.vector.tensor_copy(dst, src)
                                else:
                                    nc.scalar.copy(dst, src)
                        for o in range(O):
                            ps2 = s2psp.tile([128, nb], FP, tag="s2", name="s2")
                            for cp in (2 * q, 2 * q + 1):
                                e = cp % 2
                                for par in range(2):
                                    r = 2 * e + par
                                    nc.tensor.matmul(
                                        ps2[r * D:(r + 1) * D, :],
                                        w1t[cp][par * 64:(par + 1) * 64,
                                                o * D:(o + 1) * D],
                                        vt[par * 64:(par + 1) * 64,
                                           e * O * nb + o * nb:
                                           e * O * nb + (o + 1) * nb],
                                        tile_position=(par * 64, r * D))
                            # H0 = I - Y/theta (fp16), accumulate c_H0 term
                            hsl = hog[o][:, q * nb:(q + 1) * nb]
                            nc.vector.scalar_tensor_tensor(
                                _blk(hsl, bchunk), _blk(ps2[:, :], bchunk),
                                float(-1.0 / THETA), _bc(idt, bchunk),
                                op0=AOP.mult, op1=AOP.add)
                            nc.vector.scalar_tensor_tensor(
                                Xps[:, q * nb:(q + 1) * nb],
                                hsl, wap('H0', o),
                                Xps[:, q * nb:(q + 1) * nb],
                                op0=AOP.mult, op1=AOP.add)

                # ===== phase B: dyadic squaring chain (quad matmuls) =====
                if True:
                    for pb in (0, 2, 4):
                        hcur = [hog[pb], hog[pb + 1]]
                        for lane in range(2):
                            bbuild(btiles[lane], hcur[lane])
                        for j in range(1, K_SQ + 1):
                            for lane in range(2):
                                o = pb + lane
                                ps = wkps.tile([128, ncols], FP, tag="wk",
                                               name="wk")
                                bwave(ps, btiles[2 * ((j - 1) % 2) + lane],
                                      hcur[lane])
                                hnew = hpp.tile([128, ncols], HP, tag="hog",
                                                name="hog")
                                nc.scalar.copy(hnew[:, :], ps[:, :])
                                nc.vector.scalar_tensor_tensor(
                                    Xps[:, :], hnew[:, :], wap(f'H{j}', o),
                                    Xps[:, :], op0=AOP.mult, op1=AOP.add)
                                if j < K_SQ:
                                    bbuild(btiles[2 * (j % 2) + lane], hnew)
                                hcur[lane] = hnew

                    # const term: X += ((c0 + LOGTHETA)/8) * I
                    nc.vector.scalar_tensor_tensor(
                        _blk(Xps[:, :], nblk), _bc(idt, nblk),
                        float((HCOEF[0] + LOGTHETA) / 8.0),
                        _blk(Xps[:, :], nblk), op0=AOP.mult, op1=AOP.add)

                # ===== phase C: expm =====
                if True:
                    xs = ctp.tile([128, ncols], HP, tag="ctmp", name="ctmp")
                    nc.scalar.copy(xs[:, :], Xps[:, :])
                    bbuild(btiles[0], xs)
                    x2ps = wkps.tile([128, ncols], FP, tag="wk", name="wk")
                    bwave(x2ps, btiles[0], xs)
                    x2t = ctp.tile([128, ncols], HP, tag="ctmp", name="ctmp")
                    nc.scalar.copy(x2t[:, :], x2ps[:, :])
                    # x3 = xs @ x2 reuses the B(xs) stationary tile
                    x3ps = wkps.tile([128, ncols], FP, tag="wk", name="wk")
                    bwave(x3ps, btiles[0], x2t)
                    x3t = ctp.tile([128, ncols], HP, tag="ctmp", name="ctmp")
                    nc.scalar.copy(x3t[:, :], x3ps[:, :])
                    h1 = ctp.tile([128, ncols], HP, tag="ctmp", name="ctmp")
                    nc.vector.scalar_tensor_tensor(
                        _blk(h1[:, :], nblk), _blk(xs[:, :], nblk),
                        float(EXPC[4]), _bc(cid[3], nblk),
                        op0=AOP.mult, op1=AOP.add)
                    nc.vector.scalar_tensor_tensor(
                        h1[:, :], x2t[:, :], float(EXPC[5]), h1[:, :],
                        op0=AOP.mult, op1=AOP.add)
                    nc.vector.scalar_tensor_tensor(
                        h1[:, :], x3t[:, :], float(EXPC[6]), h1[:, :],
                        op0=AOP.mult, op1=AOP.add)
                    plow = ctp.tile([128, ncols], HP, tag="ctmp", name="ctmp")
                    nc.vector.scalar_tensor_tensor(
                        _blk(plow[:, :], nblk), _blk(xs[:, :], nblk),
                        float(EXPC[1]), _bc(cid[0], nblk),
                        op0=AOP.mult, op1=AOP.add)
                    nc.vector.scalar_tensor_tensor(
                        plow[:, :], x2t[:, :], float(EXPC[2]), plow[:, :],
                        op0=AOP.mult, op1=AOP.add)
                    bbuild(btiles[1], x3t)
                    ppps = wkps.tile([128, ncols], FP, tag="wk", name="wk")
                    bwave(ppps, btiles[1], h1)
                    e0 = ctp.tile([128, ncols], HP, tag="ctmp", name="ctmp")
                    nc.vector.scalar_tensor_tensor(
                        e0[:, :], ppps[:, :], 1.0, plow[:, :],
                        op0=AOP.mult, op1=AOP.add)
                    bbuild(btiles[2], e0)
                    e1ps = wkps.tile([128, ncols], FP, tag="wk", name="wk")
                    bwave(e1ps, btiles[2], e0)
                    e1 = ctp.tile([128, ncols], HP, tag="ctmp", name="ctmp")
                    nc.scalar.copy(e1[:, :], e1ps[:, :])
                    bbuild(btiles[3], e1)
                    e2ps = wkps.tile([128, ncols], FP, tag="wk", name="wk")
                    bwave(e2ps, btiles[3], e1)
                    e2 = ctp.tile([128, ncols], HP, tag="ctmp", name="ctmp")
                    nc.scalar.copy(e2[:, :], e2ps[:, :])
                    bbuild(btiles[0], e2)
                    e3ps = wkps.tile([128, ncols], FP, tag="wk", name="wk")
                    bwave(e3ps, btiles[0], e2)
                    outt = outp.tile([128, ncols], FP, tag="outt", name="outt")
                    nc.scalar.copy(outt[:, :], e3ps[:, :])
                    # dst AP dims match src iteration order: (r,i | b,j), per q
                    oa = out[:, :, :, :]
                    for q in range(4):
                        dst = AP(oa.tensor,
                                 ch * bchunk * C * D * D + q * 4 * D * D,
                                 [[D * D, 4], [D, D],
                                  [C * D * D, bchunk], [1, D]])
                        src = outt[:, q * nb:(q + 1) * nb].rearrange(
                            "p (b j) -> p b j", b=bchunk)
                        nc.sync.dma_start(dst, src)
    return nc


_NC_CACHE = {}


def kernel(x: np.ndarray, W: np.ndarray, weights: np.ndarray) -> np.ndarray:
    from concourse.bass_utils import run_bass_kernel_spmd
    B = x.shape[0]
    b_loc = B // NCORES
    key = (b_loc,)
    if key not in _NC_CACHE:
        nc0 = build_nc(b_loc=b_loc, bchunk=8)
        nc0.finalize()
        _NC_CACHE[key] = nc0
    nc = _NC_CACHE[key]
    wtab = host_wtab(np.asarray(weights))
    idt = host_idt()
    in_maps = [
        {"x": np.ascontiguousarray(x[i * b_loc:(i + 1) * b_loc]).astype(np.float32),
         "W": np.ascontiguousarray(W).astype(np.float32),
         "wtab": wtab, "idt": idt}
        for i in range(NCORES)
    ]
    res = run_bass_kernel_spmd(nc, in_maps, core_ids=list(range(NCORES)))
    return np.concatenate([r["out"] for r in res.results], axis=0)


# revision 3
# speedup vs baseline: 1.5436x; 1.5436x over previous
"""Trainium2 Bass kernel for nn_MixedOp_35562329211102.

Computes FM[b,c] = expm( sum_o weights[o] * logm( W[o,c]^T x[b,c] W[o,c] ) )
for x: [256,16,64,64] SPD, W: [6,16,64,32], weights: [6] (simplex).

logm via a dyadic squaring chain: H_0 = I - Y/theta, H_{j+1} = H_j^2.
log(Y) = log(theta) + log(I - H_0) ~ sum_j c_j H_j, with coefficients from a
Lawson minimax fit weighted by (1-h) -- errors at tiny eigenvalues of Y are
crushed by the final expm, so the fit spends its budget where it matters.
That lets K_SQ drop from 13 to 8 at equal end-to-end error (fp16-sim
rel_l2 ~3.8e-3 vs gate 2e-2).

Squarings run as block-diagonal quad matmuls: the stationary operand is a
128x128 block-diagonal matrix holding 4 independent 32x32 H's (one per
partition group = channel), the moving operand is the natural stacked
[128, 32] tile.  out = B^T @ D yields all 4 squares in stacked layout with
one LDWEIGHTS (128 cols -> fast-weight-load eligible) + one matmul (N=32)
instead of 4 LDW + 4 MM.  Block-diagonal operands are maintained by 3-engine
scatter copies (gpsimd/vector/scalar) into pre-zeroed persistent tiles.

expm via scaling-squaring: X = M/8, degree-6 Taylor (Paterson-Stockmeyer),
then 3 squarings, same quad-matmul scheme.

Sharding: data-parallel over batch B across 8 cores (32 batches/core).
"""

import numpy as np

import concourse.bass as bass
from concourse import bacc
import concourse.mybir as mybir
from concourse.bass import AP
from concourse.tile import TileContext

FP = mybir.dt.float32
HP = mybir.dt.float16
AOP = mybir.AluOpType

THETA = 9.0
LOGTHETA = 2.1972245773
K_SQ = 8
# (1-h)-weighted Lawson fit of log(1-h) on h in [0.0626, 0.999817]:
# log(1-h) ~ HCOEF[0] + sum_{j=0..K_SQ} HCOEF[1+j] * h^(2^j)
HCOEF = [-0.00033517, -0.99035730, -0.58854727, -0.77016997, -0.59288672,
         -0.87173622, -0.32950715, -1.43648230, 0.64457418, -2.38238148]
EXPC = [1.0, 1.0, 0.5, 1.0 / 6, 1.0 / 24, 1.0 / 120, 1.0 / 720]

C, O, D, DIN = 16, 6, 32, 64
NCORES = 8

WT_KINDS = [f'H{j}' for j in range(K_SQ + 1)]
WT_NCOL = len(WT_KINDS) * O


def host_wtab(weights: np.ndarray) -> np.ndarray:
    """[128, WT_NCOL] per-partition scalar table: w[o]/8 * c_j."""
    w8 = weights.astype(np.float64) / 8.0
    cols = [w8 * HCOEF[1 + j] for j in range(K_SQ + 1)]
    row = np.concatenate(cols)
    return np.tile(row[None, :], (128, 1)).astype(np.float32)


def host_idt() -> np.ndarray:
    """[128, 32]: 4 stacked 32x32 identities."""
    return np.tile(np.eye(D, dtype=np.float32), (4, 1))


def _bc(t, nblk):
    """broadcast a [128, D] tile AP over nblk column blocks -> [128, nblk, D]."""
    a = t[:, :]
    return AP(a.tensor, a.offset, [list(a.ap[0]), [0, nblk], [1, D]])


def _blk(ap, nblk):
    """view a [128, nblk*D] AP as [128, nblk, D]."""
    return ap.rearrange("p (n j) -> p n j", n=nblk)


def build_nc(b_loc=32, bchunk=8, replicate=1):
    nchunk = b_loc // bchunk
    nb = bchunk * D          # stage2 N per (o,c)
    ncols = 4 * bchunk * D   # stacked tile width (32 col-blocks of 32)
    nblk = 4 * bchunk        # 32x32 col-blocks per stacked tile
    bcols = nblk * 128       # block-diag tile width

    nc = bacc.Bacc("TRN2")
    x = nc.dram_tensor("x", [b_loc, C, DIN, DIN], FP, kind="ExternalInput")
    Wt = nc.dram_tensor("W", [O, C, DIN, D], FP, kind="ExternalInput")
    wtab_d = nc.dram_tensor("wtab", [128, WT_NCOL], FP, kind="ExternalInput")
    idt_d = nc.dram_tensor("idt", [128, D], FP, kind="ExternalInput")
    out = nc.dram_tensor("out", [b_loc, C, D, D], FP, kind="ExternalOutput")

    with TileContext(nc) as tc, (
        tc.tile_pool(name="consts", bufs=1)) as consts, (
        tc.tile_pool(name="xp", bufs=4)) as xp, (
        tc.tile_pool(name="vp", bufs=2)) as vp, (
        tc.tile_pool(name="hp", bufs=10)) as hpp, (
        tc.tile_pool(name="ct", bufs=7)) as ctp, (
        tc.tile_pool(name="outp", bufs=2)) as outp, (
        tc.tile_pool(name="xaccp", bufs=2)) as xaccp, (
        tc.tile_pool(name="s1ps", bufs=1, space="PSUM")) as s1psp, (
        tc.tile_pool(name="s2ps", bufs=1, space="PSUM")) as s2psp, (
        tc.tile_pool(name="wkps", bufs=3, space="PSUM")) as wkps:

        # ---- constants ----
        w1t = []
        for cp in range(C // 2):
            tf = consts.tile([128, O * D], FP, tag=f"w1f_{cp}")
            for e in range(2):
                dst = tf[64 * e:64 * (e + 1), :].rearrange("p (o j) -> p o j", o=O)
                src = Wt[:, 2 * cp + e, :, :].rearrange("o p j -> p o j")
                nc.sync.dma_start(dst, src)
            th = consts.tile([128, O * D], HP, tag=f"w1_{cp}")
            nc.vector.tensor_copy(th[:, :], tf[:, :])
            w1t.append(th)
        wtab = consts.tile([128, WT_NCOL], FP, tag="wtab", name="wtab")
        nc.sync.dma_start(wtab[:, :], wtab_d[:, :])
        idt = consts.tile([128, D], FP, tag="idt", name="idt")
        nc.sync.dma_start(idt[:, :], idt_d[:, :])
        cid = {}
        for k in (0, 3):
            t = consts.tile([128, D], HP, tag=f"cid{k}")
            nc.vector.tensor_scalar_mul(t[:, :], idt[:, :], float(EXPC[k]))
            cid[k] = t

        # persistent block-diagonal stationary tiles (4 slots), pre-zeroed
        # once; scatter copies only ever write the diagonal 32x32 blocks, so
        # the off-diagonal zeros persist across reuse.
        btiles = []
        for s in range(4):
            bt = consts.tile([128, bcols], HP, tag=f"bdiag{s}", name=f"bdiag{s}")
            nc.gpsimd.memset(bt[:, :], 0.0)
            btiles.append(bt)

        def wap(kind, o):
            i = WT_KINDS.index(kind) * O + o
            return wtab[:, i:i + 1]

        def bbuild(bt, src):
            """scatter stacked [128, ncols] -> diag blocks of [128, bcols].

            Done with SBUF->SBUF DMA: compute engines pay ~100ns per strided
            run (32 runs of 64B here -> ~3.5us/copy), DMA handles strided
            descriptors at line rate on its own SBUF port."""
            for g in range(4):
                s = src[32 * g:32 * (g + 1), :].rearrange(
                    "p (n j) -> p n j", n=nblk)
                da = bt[32 * g:32 * (g + 1), :]
                dst = AP(da.tensor, da.offset + 32 * g,
                         [list(da.ap[0]), [128, nblk], [1, 32]])
                nc.sync.dma_start(dst, s)

        def bwave(ps, bt, src):
            """ps[:, blk*32:+32] = bt[:, blk*128:+128]^T @ src[:, blk*32:+32]
            for all col-blocks: 4 squares per matmul via block-diag weights."""
            for blk in range(nblk):
                nc.tensor.matmul(ps[:, blk * 32:(blk + 1) * 32],
                                 bt[:, blk * 128:(blk + 1) * 128],
                                 src[:, blk * 32:(blk + 1) * 32])

        for _rep in range(replicate):
          for ch in range(nchunk):
            if True:
                Xps = xaccp.tile([128, ncols], FP, tag="xacc", name="xacc")
                nc.vector.memset(Xps[:, :], 0.0)
                hog = [hpp.tile([128, ncols], HP, tag="hog", name="hog")
                       for _ in range(O)]

                # ===== phase A: BiMap + H0 =====
                if True:
                    for q in range(4):
                        vt = vp.tile([128, 2 * O * nb], HP, tag="v", name="v")
                        for cp in (2 * q, 2 * q + 1):
                            e = cp % 2
                            # one bulk DMA + cast for all bchunk batches
                            xf = xp.tile([128, bchunk * DIN], FP, tag="xf",
                                         name="xf")
                            xa = x[:, :, :, :]
                            xsrc = AP(
                                xa.tensor,
                                (ch * bchunk) * C * DIN * DIN
                                + 2 * cp * DIN * DIN,
                                [[DIN * DIN, 2], [DIN, DIN],
                                 [C * DIN * DIN, bchunk], [1, DIN]])
                            nc.sync.dma_start(
                                xf[:, :].rearrange("p (b j) -> p b j",
                                                   b=bchunk), xsrc)
                            xt = xp.tile([128, bchunk * DIN], HP, tag="xt",
                                         name="xt")
                            nc.scalar.copy(xt[:, :], xf[:, :])
                            for bb in range(bchunk):
                                ps1 = s1psp.tile([128, O * D], FP, tag="s1",
                                                 name="s1")
                                xs_ = xt[:, bb * DIN:(bb + 1) * DIN]
                                nc.tensor.matmul(ps1[0:64, :], xs_[0:64, :],
                                                 w1t[cp][0:64, :],
                                                 tile_position=(0, 0))
                                nc.tensor.matmul(ps1[64:128, :], xs_[64:128, :],
                                                 w1t[cp][64:128, :],
                                                 tile_position=(64, 64))
                                # scatter V into o-major layout
                                src = ps1[:, :].rearrange("p (o j) -> p o j", o=O)
                                va = vt[:, :]
                                dst = AP(va.tensor,
                                         va.offset + e * O * nb + bb * D,
                                         [list(va.ap[0]), [nb, O], [1, D]])
                                if bb % 2 == 0:
                                    nc.vector.tensor_copy(dst, src)
                                else:
                                    nc.scalar.copy(dst, src)
                        for o in range(O):
                            ps2 = s2psp.tile([128, nb], FP, tag="s2", name="s2")
                            for cp in (2 * q, 2 * q + 1):
                                e = cp % 2
                                for par in range(2):
                                    r = 2 * e + par
                                    nc.tensor.matmul(
                                        ps2[r * D:(r + 1) * D, :],
                                        w1t[cp][par * 64:(par + 1) * 64,
                                                o * D:(o + 1) * D],
                                        vt[par * 64:(par + 1) * 64,
                                           e * O * nb + o * nb:
                                           e * O * nb + (o + 1) * nb],
                                        tile_position=(par * 64, r * D))
                            # H0 = I - Y/theta (fp16), accumulate c_H0 term
                            hsl = hog[o][:, q * nb:(q + 1) * nb]
                            nc.vector.scalar_tensor_tensor(
                                _blk(hsl, bchunk), _blk(ps2[:, :], bchunk),
                                float(-1.0 / THETA), _bc(idt, bchunk),
                                op0=AOP.mult, op1=AOP.add)
                            nc.vector.scalar_tensor_tensor(
                                Xps[:, q * nb:(q + 1) * nb],
                                hsl, wap('H0', o),
                                Xps[:, q * nb:(q + 1) * nb],
                                op0=AOP.mult, op1=AOP.add)

                # ===== phase B: dyadic squaring chain (quad matmuls) =====
                if True:
                    for pb in (0, 2, 4):
                        hcur = [hog[pb], hog[pb + 1]]
                        for lane in range(2):
                            bbuild(btiles[lane], hcur[lane])
                        for j in range(1, K_SQ + 1):
                            for lane in range(2):
                                o = pb + lane
                                ps = wkps.tile([128, ncols], FP, tag="wk",
                                               name="wk")
                                bwave(ps, btiles[2 * ((j - 1) % 2) + lane],
                                      hcur[lane])
                                hnew = hpp.tile([128, ncols], HP, tag="hog",
                                                name="hog")
                                nc.scalar.copy(hnew[:, :], ps[:, :])
                                nc.vector.scalar_tensor_tensor(
                                    Xps[:, :], hnew[:, :], wap(f'H{j}', o),
                                    Xps[:, :], op0=AOP.mult, op1=AOP.add)
                                if j < K_SQ:
                                    bbuild(btiles[2 * (j % 2) + lane], hnew)
                                hcur[lane] = hnew

                    # const term: X += ((c0 + LOGTHETA)/8) * I
                    nc.vector.scalar_tensor_tensor(
                        _blk(Xps[:, :], nblk), _bc(idt, nblk),
                        float((HCOEF[0] + LOGTHETA) / 8.0),
                        _blk(Xps[:, :], nblk), op0=AOP.mult, op1=AOP.add)

                # ===== phase C: expm =====
                if True:
                    xs = ctp.tile([128, ncols], HP, tag="ctmp", name="ctmp")
                    nc.scalar.copy(xs[:, :], Xps[:, :])
                    bbuild(btiles[0], xs)
                    x2ps = wkps.tile([128, ncols], FP, tag="wk", name="wk")
                    bwave(x2ps, btiles[0], xs)
                    x2t = ctp.tile([128, ncols], HP, tag="ctmp", name="ctmp")
                    nc.scalar.copy(x2t[:, :], x2ps[:, :])
                    # x3 = xs @ x2 reuses the B(xs) stationary tile
                    x3ps = wkps.tile([128, ncols], FP, tag="wk", name="wk")
                    bwave(x3ps, btiles[0], x2t)
                    x3t = ctp.tile([128, ncols], HP, tag="ctmp", name="ctmp")
                    nc.scalar.copy(x3t[:, :], x3ps[:, :])
                    h1 = ctp.tile([128, ncols], HP, tag="ctmp", name="ctmp")
                    nc.vector.scalar_tensor_tensor(
                        _blk(h1[:, :], nblk), _blk(xs[:, :], nblk),
                        float(EXPC[4]), _bc(cid[3], nblk),
                        op0=AOP.mult, op1=AOP.add)
                    nc.vector.scalar_tensor_tensor(
                        h1[:, :], x2t[:, :], float(EXPC[5]), h1[:, :],
                        op0=AOP.mult, op1=AOP.add)
                    nc.vector.scalar_tensor_tensor(
                        h1[:, :], x3t[:, :], float(EXPC[6]), h1[:, :],
                        op0=AOP.mult, op1=AOP.add)
                    plow = ctp.tile([128, ncols], HP, tag="ctmp", name="ctmp")
                    nc.vector.scalar_tensor_tensor(
                        _blk(plow[:, :], nblk), _blk(xs[:, :], nblk),
                        float(EXPC[1]), _bc(cid[0], nblk),
                        op0=AOP.mult, op1=AOP.add)
                    nc.vector.scalar_tensor_tensor(
                        plow[:, :], x2t[:, :], float(EXPC[2]), plow[:, :],
                        op0=AOP.mult, op1=AOP.add)
                    bbuild(btiles[1], x3t)
                    ppps = wkps.tile([128, ncols], FP, tag="wk", name="wk")
                    bwave(ppps, btiles[1], h1)
                    e0 = ctp.tile([128, ncols], HP, tag="ctmp", name="ctmp")
                    nc.vector.scalar_tensor_tensor(
                        e0[:, :], ppps[:, :], 1.0, plow[:, :],
                        op0=AOP.mult, op1=AOP.add)
                    bbuild(btiles[2], e0)
                    e1ps = wkps.tile([128, ncols], FP, tag="wk", name="wk")
                    bwave(e1ps, btiles[2], e0)
                    e1 = ctp.tile([128, ncols], HP, tag="ctmp", name="ctmp")
                    nc.scalar.copy(e1[:, :], e1ps[:, :])
                    bbuild(btiles[3], e1)
                    e2ps = wkps.tile([128, ncols], FP, tag="wk", name="wk")
                    bwave(e2ps, btiles[3], e1)
                    e2 = ctp.tile([128, ncols], HP, tag="ctmp", name="ctmp")
                    nc.scalar.copy(e2[:, :], e2ps[:, :])
                    bbuild(btiles[0], e2)
                    e3ps = wkps.tile([128, ncols], FP, tag="wk", name="wk")
                    bwave(e3ps, btiles[0], e2)
                    outt = outp.tile([128, ncols], FP, tag="outt", name="outt")
                    nc.scalar.copy(outt[:, :], e3ps[:, :])
                    # dst AP dims match src iteration order: (r,i | b,j), per q
                    oa = out[:, :, :, :]
                    for q in range(4):
                        dst = AP(oa.tensor,
                                 ch * bchunk * C * D * D + q * 4 * D * D,
                                 [[D * D, 4], [D, D],
                                  [C * D * D, bchunk], [1, D]])
                        src = outt[:, q * nb:(q + 1) * nb].rearrange(
                            "p (b j) -> p b j", b=bchunk)
                        nc.sync.dma_start(dst, src)
    return nc


_NC_CACHE = {}


def kernel(x: np.ndarray, W: np.ndarray, weights: np.ndarray) -> np.ndarray:
    from concourse.bass_utils import run_bass_kernel_spmd
    B = x.shape[0]
    b_loc = B // NCORES
    key = (b_loc,)
    if key not in _NC_CACHE:
        nc0 = build_nc(b_loc=b_loc, bchunk=8)
        nc0.finalize()
        _NC_CACHE[key] = nc0
    nc = _NC_CACHE[key]
    wtab = host_wtab(np.asarray(weights))
    idt = host_idt()
    in_maps = [
        {"x": np.ascontiguousarray(x[i * b_loc:(i + 1) * b_loc]).astype(np.float32),
         "W": np.ascontiguousarray(W).astype(np.float32),
         "wtab": wtab, "idt": idt}
        for i in range(NCORES)
    ]
    res = run_bass_kernel_spmd(nc, in_maps, core_ids=list(range(NCORES)))
    return np.concatenate([r["out"] for r in res.results], axis=0)
